# revision 24
# baseline (speedup 1.0000x reference)
"""DAGCN kernel for Trainium2, 8 NeuronCores, sharded over the T (time) axis.

Math (per time step t):
  A      = relu(E_t E_t^T)   (symmetric)
  PU     = exp(A)            (softmax numerator; A <= ~20 so no overflow)
  inv[n] = 1 / sum_s PU[n, s]  ( = 1/colsum_m PU[m, n] by symmetry)
  S[n,m] = PU[n,m] inv[n];  d[n] = S[n,n] = exp(E[n]^2) inv[n]
  xg1    = S @ x
  out[b,n,o] = sum_i x[b,n,i] V0[n,i,o] + sum_i xg1[b,n,i] V1[n,i,o] + bias[n,o]
     where V0 = W0 - W2,  V1 = W1 + 2 d_n W2   (Chebyshev fold, K=3)

Layout strategy per core:
  - W loaded in NATURAL layout [n-part, (k i o)] (48KB contiguous runs -> full
    DMA bandwidth, f32->bf16 converted by the Pool SWDGE), V0/V1 combined
    elementwise there (per-n scalar 2d works since n is the partition), then
    rotated to [(j i), o, n] with per-o PE transposes.
  - xg1^T produced DIRECTLY by PE: psum[(b2,i), n] = sum_m x[m,(b2,i)] S'T[m,n]
    with S'T[m,n] = PU[m,n] inv[n], pair-batching two b's per psum so the S'T
    moving stream is shared.  x^T via bulk PE transposes with the same pair
    layout.  Partition-half fixups are done with cheap SBUF->SBUF DMAs.
  - e2: one matmul per n: psum[o, b'] = [V0;V1]^T_n @ STK[:, n, :], bias added
    on copy-out, PE-transposed back to [n-part, b, o], bf16->f32 converting
    store in natural b-major DRAM order (b' = parity-major permutation).
"""
import sys

sys.path.insert(0, "/opt/trn_rl_repo")
import numpy as np

B, T, N, C, O, K = 32, 24, 512, 64, 64, 3
NCORES = 8
T_LOC = T // NCORES  # 3 time steps per core

_CACHE = {}


def build_bass(variant="full"):
    if variant in _CACHE:
        return _CACHE[variant]
    from contextlib import ExitStack

    import concourse.bass as bass
    import concourse.mybir as mybir
    from concourse import bacc
    import concourse.tile as tile
    from concourse.bass import ts
    from concourse.masks import make_identity

    f32 = mybir.dt.float32
    f32r = mybir.dt.float32r
    bf16 = mybir.dt.bfloat16
    Alu = mybir.AluOpType
    Act = mybir.ActivationFunctionType
    AX = mybir.AxisListType.X

    nc = bacc.Bacc()
    x_d = nc.dram_tensor("x_sh", [B, T_LOC, N, C], f32, kind="ExternalInput")
    e_d = nc.dram_tensor("emb_sh", [T_LOC, N], f32, kind="ExternalInput")
    w_d = nc.dram_tensor("w_sh", [T_LOC, N, K, C, O], f32, kind="ExternalInput")
    b_d = nc.dram_tensor("bias_sh", [T_LOC, N, O], f32, kind="ExternalInput")
    o_d = nc.dram_tensor("out_sh", [B, T_LOC, N, O], f32, kind="ExternalOutput")

    with tile.TileContext(nc) as tc, ExitStack() as ctx:
        p1 = ctx.enter_context(tc.tile_pool(name="singles", bufs=1))
        p_pu = ctx.enter_context(tc.tile_pool(name="pu", bufs=1))
        p_sm = ctx.enter_context(tc.tile_pool(name="sm", bufs=2))
        p_sm1 = ctx.enter_context(tc.tile_pool(name="sm1", bufs=1))
        p_st = ctx.enter_context(tc.tile_pool(name="st", bufs=1))
        p_x = ctx.enter_context(tc.tile_pool(name="xin", bufs=2))
        p_stk = ctx.enter_context(tc.tile_pool(name="stk", bufs=1))
        p_tmp = ctx.enter_context(tc.tile_pool(name="tmp", bufs=1))
        p_w = ctx.enter_context(tc.tile_pool(name="wt", bufs=2))
        p_v = ctx.enter_context(tc.tile_pool(name="vt", bufs=1))
        p_vt = ctx.enter_context(tc.tile_pool(name="vtt", bufs=1))
        p_bt = ctx.enter_context(tc.tile_pool(name="bt", bufs=2))
        p_ot = ctx.enter_context(tc.tile_pool(name="ot", bufs=2))
        p_ob = ctx.enter_context(tc.tile_pool(name="ob", bufs=1))
        # PSUM pools (16KB/partition total budget)
        ps_e1 = ctx.enter_context(tc.tile_pool(name="pse1", bufs=2, space="PSUM"))
        ps_xtp = ctx.enter_context(tc.tile_pool(name="psxt", bufs=1, space="PSUM"))
        ps_r = ctx.enter_context(tc.tile_pool(name="psr", bufs=3, space="PSUM"))
        ps_e2 = ctx.enter_context(tc.tile_pool(name="pse2", bufs=2, space="PSUM"))

        identb = p1.tile([128, 128], bf16)
        make_identity(nc, identb)
        ones128_f = p1.tile([128, 128], f32)
        nc.vector.memset(ones128_f, 1.0)
        ones128 = p1.tile([128, 128], f32r)
        nc.vector.tensor_copy(out=ones128[:], in_=ones128_f[:])

        n_rep = 4 if variant == "fullx4" else 1
        for tt in range(T_LOC * n_rep):
            t = tt % T_LOC
            # ---------------- scores phase ----------------
            e_row_f = p_sm1.tile([1, 512], f32, tag="erow")
            nc.sync.dma_start(out=e_row_f, in_=e_d[t][None, :])
            e_row = p_sm1.tile([1, 512], f32r, tag="erowr")
            nc.vector.tensor_copy(out=e_row[:], in_=e_row_f[:])
            e_col = p_sm1.tile([128, 4], f32, tag="ecol")
            nc.sync.dma_start(out=e_col, in_=e_d[t].rearrange("(c p) -> p c", p=128))

            pus, invcols = [], []
            for mc in range(4):
                ps = ps_e1.tile([128, 512], f32, tag="g")
                nc.tensor.matmul(ps[:], e_row[:, ts(mc, 128)], e_row[:],
                                 start=True, stop=True)
                a_sb = p_sm.tile([128, 512], f32, tag="a")
                nc.vector.tensor_single_scalar(a_sb[:], ps[:], 0.0, Alu.max)  # relu
                pu = p_pu.tile([128, 512], f32r, tag=f"pu{mc}")
                nc.scalar.activation(pu[:], a_sb[:], Act.Exp)
                pus.append(pu)
                # row-sum -> inv column (for d); symmetric == col-sum
                rs = p_sm.tile([128, 1], f32, tag="rs")
                nc.vector.tensor_reduce(rs[:], pu[:], axis=AX, op=Alu.add)
                invc = p_sm.tile([128, 1], f32, tag=f"invc{mc}")
                nc.vector.reciprocal(invc[:], rs[:])
                invcols.append(invc)

            # colsum broadcast to all partitions via ones-matrix matmul
            ps_b = ps_e1.tile([128, 512], f32, tag="g")
            for mc in range(4):
                nc.tensor.matmul(ps_b[:], ones128[:], pus[mc][:],
                                 start=(mc == 0), stop=(mc == 3))
            invb = p_sm.tile([128, 512], f32, tag="invb")
            nc.vector.reciprocal(invb[:], ps_b[:])

            sts = []
            for mc in range(4):
                st = p_st.tile([128, 512], bf16, tag=f"st{mc}")
                nc.vector.tensor_tensor(out=st[:], in0=pus[mc][:], in1=invb[:],
                                        op=Alu.mult)
                sts.append(st)

            # 2*d columns per n-block: 2 * exp(e^2) * inv
            d2s = []
            for nb in range(4):
                sq = p_sm.tile([128, 1], f32, tag="sq")
                nc.vector.tensor_mul(sq[:], e_col[:, nb:nb + 1], e_col[:, nb:nb + 1])
                esq = p_sm.tile([128, 1], f32, tag="esq")
                nc.scalar.activation(esq[:], sq[:], Act.Exp)
                t1 = p_sm.tile([128, 1], f32, tag="t1")
                nc.vector.tensor_mul(t1[:], esq[:], invcols[nb][:])
                d2 = p_sm.tile([128, 1], f32, tag=f"d2{nb}")
                nc.vector.tensor_add(d2[:], t1[:], t1[:])
                d2s.append(d2)

            # ---------------- x load ----------------
            do_stack = variant in ("full", "fullx4", "stack", "noe2") or (
                variant == "nostack_e2" and t == 0)
            do_nb = variant in ("full", "fullx4", "wrot", "noe2", "nostack_e2")
            do_e2 = variant in ("full", "fullx4", "nostack_e2")
            xts = []
            wts = [None] * 4
            if variant == "stack" or do_stack:
                for mc in range(4):
                    xt = p_x.tile([128, 32, 64], bf16, tag=f"x{mc}")
                    nc.gpsimd.dma_start(
                        out=xt,
                        in_=x_d[:, t, ts(mc, 128), :].rearrange("b m c -> m b c"))
                    xts.append(xt)
            if do_nb:
                wt0 = p_w.tile([128, 3, 64, 64], bf16, tag="w")
                nc.gpsimd.dma_start(out=wt0, in_=w_d[t, ts(0, 128)])
                wts[0] = wt0

            # ------- stack build: STK[(j i), parity, pair, n] -------
            # j=0 (partitions 0:64)   = x^T    -> pairs with V0
            # j=1 (partitions 64:128) = xg1^T  -> pairs with V1
            if t == 0 or variant != "nostack_e2":
                stk = p_stk.tile([128, 2, 16, 512], bf16, tag="stk")
            tmp = p_tmp.tile([128, 16, 512], bf16, tag="tmp")
            for pr in range(16 if do_stack else 0):
                pair = [xts[mc][:, 2 * pr:2 * pr + 2, :] for mc in range(4)]
                ps_g = ps_e1.tile([128, 512], f32, tag="g")
                for mc in range(4):
                    nc.tensor.matmul(ps_g[:], pair[mc], sts[mc][:],
                                     start=(mc == 0), stop=(mc == 3))
                ps_x = ps_xtp.tile([128, 512], bf16, tag="xT")
                for mc in range(4):
                    nc.tensor.transpose(ps_x[:, ts(mc, 128)], pair[mc], identb[:])
                # direct halves (partition-preserving)
                nc.vector.tensor_copy(out=stk[64:128, 1, pr, :], in_=ps_g[64:128, :])
                nc.scalar.activation(out=stk[0:64, 0, pr, :], in_=ps_x[0:64, :],
                                     func=Act.Copy)
                # staged halves (partition shift via DMA afterwards)
                nc.vector.tensor_copy(out=tmp[0:64, pr, :], in_=ps_g[0:64, :])
                nc.scalar.activation(out=tmp[64:128, pr, :], in_=ps_x[64:128, :],
                                     func=Act.Copy)
            # partition-shifting SBUF->SBUF DMAs
            if do_stack:
                nc.gpsimd.dma_start(out=stk[64:128, 0, :, :], in_=tmp[0:64, :, :])
                nc.gpsimd.dma_start(out=stk[0:64, 1, :, :], in_=tmp[64:128, :, :])

            # ------- per n-block: W load, V fold, rotate, e2, out -------
            for nb in range(4 if do_nb else 0):
                wt = wts[nb]
                if nb + 1 < 4:
                    wt_nxt = p_w.tile([128, 3, 64, 64], bf16, tag="w")
                    nc.gpsimd.dma_start(out=wt_nxt, in_=w_d[t, ts(nb + 1, 128)])
                    wts[nb + 1] = wt_nxt
                v = p_v.tile([128, 2, 64, 64], bf16, tag="v")
                # V0 = W0 - W2 ; V1 = 2d*W2 + W1
                nc.gpsimd.tensor_tensor(
                    out=v[:, 0].rearrange("p i o -> p (i o)"),
                    in0=wt[:, 0].rearrange("p i o -> p (i o)"),
                    in1=wt[:, 2].rearrange("p i o -> p (i o)"), op=Alu.subtract)
                nc.vector.scalar_tensor_tensor(
                    out=v[:, 1].rearrange("p i o -> p (i o)"),
                    in0=wt[:, 2].rearrange("p i o -> p (i o)"),
                    scalar=d2s[nb][:],
                    in1=wt[:, 1].rearrange("p i o -> p (i o)"),
                    op0=Alu.mult, op1=Alu.add)

                # rotate: VT[(j i), o, n] ; partition p = j*64+i
                vt = p_vt.tile([128, 64, 128], bf16, tag="vt")
                for og in range(8):
                    ps_rt = ps_r.tile([128, 8, 128], bf16, tag="r")
                    for q in range(8):
                        o_idx = og * 8 + q
                        nc.tensor.transpose(ps_rt[:, q, :], v[:, :, :, o_idx],
                                            identb[:])
                    dst = vt[:, og * 8:og * 8 + 8, :].rearrange("p o n -> p (o n)")
                    src = ps_rt[:].rearrange("p o n -> p (o n)")
                    if og % 2 == 0:
                        nc.scalar.activation(out=dst, in_=src, func=Act.Copy)
                    else:
                        nc.vector.tensor_copy(out=dst, in_=src)

                # bias^T: [64 o, 128 n]
                if not do_e2:
                    continue
                bias_in = p_bt.tile([128, 64], f32, tag="bin")
                nc.sync.dma_start(out=bias_in, in_=b_d[t, ts(nb, 128), :])
                # e2: per n one matmul  psum[o, b'] with stationary [V0; V1]
                outT = p_ot.tile([64, 128, 32], bf16, tag="outT")
                for ng in range(8):
                    ps_o = ps_e2.tile([64, 16, 32], f32, tag="o")
                    for j in range(16):
                        n_loc = ng * 16 + j
                        nc.tensor.matmul(
                            ps_o[:, j, :], vt[:, :, n_loc],
                            stk[:, :, :, nb * 128 + n_loc],
                            start=True, stop=True)
                    dst = outT[:, ng * 16:ng * 16 + 16, :]
                    nc.scalar.activation(out=dst, in_=ps_o[:], func=Act.Copy)

                # back-transpose to [n, b', o] and store (b' = parity-major)
                out_sb = p_ob.tile([128, 2, 16, 64], bf16, tag="osb")
                osb_flat = out_sb.rearrange("p two h o -> p (two h) o")
                for g in range(8):
                    ps_qt = ps_r.tile([128, 4, 128], bf16, tag="r")
                    ps_q = ps_qt[:, :, 0:64]
                    for q in range(4):
                        bp = g * 4 + q
                        nc.tensor.transpose(ps_q[:, q, :], outT[:, :, bp],
                                            identb[0:64, 0:64])
                    nc.vector.tensor_tensor(
                        out=osb_flat[:, g * 4:g * 4 + 4, :],
                        in0=ps_q,
                        in1=bias_in.unsqueeze(1).broadcast_to([128, 4, 64]),
                        op=Alu.add)
                o_view = o_d[:, t, ts(nb, 128), :].rearrange(
                    "(h two) n o -> n two h o", two=2)
                nc.gpsimd.dma_start(out=o_view[:, 0], in_=out_sb[:, 0])
                nc.gpsimd.dma_start(out=o_view[:, 1], in_=out_sb[:, 1])

        if variant not in ("full", "fullx4"):
            # token output write so the module has a produced ExternalOutput
            nc.gpsimd.dma_start(out=o_d[0, 0, 0, :][None, :],
                                in_=identb[0:1, 0:64])

    nc.finalize()
    _CACHE[variant] = nc
    return nc


def run_spmd(inputs, **kwargs):
    from concourse.bass_utils import run_bass_kernel_spmd

    x = np.ascontiguousarray(inputs["x"], dtype=np.float32)
    emb = np.ascontiguousarray(inputs["dn_embeddings"], dtype=np.float32)
    w = np.ascontiguousarray(inputs["weights_pool"], dtype=np.float32)
    bias = np.ascontiguousarray(inputs["bias_pool"], dtype=np.float32)

    nc = build_bass()
    in_maps = []
    for c in range(NCORES):
        sl = slice(c * T_LOC, (c + 1) * T_LOC)
        in_maps.append({
            "x_sh": np.ascontiguousarray(x[:, sl]),
            "emb_sh": np.ascontiguousarray(emb[sl]),
            "w_sh": np.ascontiguousarray(w[sl]),
            "bias_sh": np.ascontiguousarray(bias[sl]),
        })
    res = run_bass_kernel_spmd(nc, in_maps, core_ids=list(range(NCORES)), **kwargs)
    out = np.concatenate([r["out_sh"] for r in res.results], axis=1)
    return out, res


def kernel(**inputs):
    out, _ = run_spmd(inputs)
    return out
